# revision 18
# baseline (speedup 1.0000x reference)
"""2-layer LSTM (B=2048, S=512, H=64) + final FC on Trainium2, batch-sharded
across 8 NeuronCores (256 batch per core).

Per-core layout:
  - State z = [h0; h1] and s = [c0; c1] as [128, 256] SBUF tiles
    (partition = stacked layer0/layer1 hidden, free = local batch).
  - Tick t computes layer0 step t and layer1 step t-1 (1-tick skew), so both
    layers' gates come from one pair of matmuls per gate group.
  - Gates PSUM tile [128, 1024] = [i | f | o | g] x 256 batch columns; sigmoid
    runs as ONE activation over cols 0:768, tanh over 768:1024.
  - x_t (input size 1) and the biases are folded into a K=2 matmul against an
    aux tile [x_t; 1] DMA'd from DRAM each tick.
"""

import numpy as np
import concourse.bass as bass
import concourse.mybir as mybir
from concourse import bacc
from concourse.tile import TileContext
from concourse import bass_utils

HIDDEN = 64
OUTPUT = 12
B = 2048
NCORES = 8
BL = B // NCORES  # 256 local batch

F32 = mybir.dt.float32
AFT = mybir.ActivationFunctionType

# gate-group order in PSUM columns: [i, f, o, g]; pytorch rows are i,f,g,o
GATE_SLICES = [(0, 64), (64, 128), (192, 256), (128, 192)]  # i, f, o, g

_BUILD_CACHE = {}


def _build(nticks: int) -> bass.Bass:
    nc = bacc.Bacc()
    xT = nc.dram_tensor("xT", [nticks, 2, BL], F32, kind="ExternalInput")
    # packed consts: [:,0:512]=WA, rows0:2 of 512:1024=AUXW (x-weights only),
    # rows64:128 of 1024:1036=FCW(T), row0 of 1036:1048=FCB,
    # row0 of 1048:1560=per-gate-group bias rows
    CONST = nc.dram_tensor("CONST", [128, 1560], F32, kind="ExternalInput")
    OUT = nc.dram_tensor("out", [BL, OUTPUT], F32, kind="ExternalOutput")

    with TileContext(nc) as tc:
        with (
            tc.tile_pool(name="const", bufs=1) as cpool,
            tc.tile_pool(name="state", bufs=3) as spool,
            tc.tile_pool(name="work", bufs=3) as wpool,
            tc.tile_pool(name="aux", bufs=4) as apool,
            tc.tile_pool(name="ps", bufs=2, space="PSUM") as pspool,
            tc.tile_pool(name="psfc", bufs=1, space="PSUM") as fpool,
        ):
            cst = cpool.tile([128, 1560], F32, tag="cst")
            nc.gpsimd.dma_start(cst[:], CONST[:])
            wa = cst[:, 0:512]
            auxw = cst[0:2, 512:1024]
            fcw = cst[64:128, 1024:1036]
            fcb = cst[0:1, 1036:1048]
            biasw = cst[0:1, 1048:1560]
            ones = cpool.tile([1, BL], F32, tag="ones")
            nc.vector.memset(ones[:], 1.0)

            z = spool.tile([128, BL], F32, tag="z")
            nc.vector.memset(z[:], 0.0)
            s = spool.tile([128, BL], F32, tag="s")
            nc.vector.memset(s[:], 0.0)

            for t in range(nticks):
                auxt = apool.tile([2, BL], F32, tag="aux")
                nc.gpsimd.dma_start(auxt[:], xT[t])

                ps = pspool.tile([128, 1024], F32, tag="ps")
                for X in range(4):
                    c0, c1 = X * 256, (X + 1) * 256
                    # const-only first writer: absorbs the PSUM-slot WAR/WAW
                    # waits so the z/aux matmuls stay under the 2-wait cap
                    nc.tensor.matmul(
                        ps[:, c0:c1], biasw[:, X * 128 : (X + 1) * 128], ones[:],
                        start=True, stop=False,
                    )
                    nc.tensor.matmul(
                        ps[:, c0:c1], wa[:, X * 128 : (X + 1) * 128], z[:],
                        start=False, stop=False,
                    )
                    nc.tensor.matmul(
                        ps[:, c0:c1], auxw[:, X * 128 : (X + 1) * 128], auxt[:],
                        start=False, stop=True,
                    )

                tifo = wpool.tile([128, 768], F32, tag="tifo")
                nc.scalar.activation(tifo[:], ps[:, 0:768], AFT.Sigmoid)
                tg = wpool.tile([128, BL], F32, tag="tg")
                nc.scalar.activation(tg[:], ps[:, 768:1024], AFT.Tanh)

                ig = wpool.tile([128, BL], F32, tag="ig")
                nc.vector.tensor_mul(ig[:], tifo[:, 0:256], tg[:])
                fc = wpool.tile([128, BL], F32, tag="fc")
                nc.vector.tensor_mul(fc[:], tifo[:, 256:512], s[:])
                s = spool.tile([128, BL], F32, tag="s")
                nc.vector.tensor_add(s[:], ig[:], fc[:])
                tch = wpool.tile([128, BL], F32, tag="tch")
                nc.scalar.activation(tch[:], s[:], AFT.Tanh)
                z = spool.tile([128, BL], F32, tag="z")
                nc.vector.tensor_mul(z[:], tifo[:, 512:768], tch[:])

                if t == 0:
                    # layer1 "step -1" output is junk; reset its state to 0
                    nc.vector.memset(z[64:128, :], 0.0)
                    nc.vector.memset(s[64:128, :], 0.0)

            for half in range(2):
                psf = fpool.tile([128, OUTPUT], F32, tag="psfc")
                nc.tensor.matmul(
                    psf[:], z[64:128, half * 128 : (half + 1) * 128], fcw,
                    start=True, stop=False,
                )
                nc.tensor.matmul(psf[:], ones[:, 0:128], fcb[:], start=False, stop=True)
                ob = wpool.tile([128, OUTPUT], F32, tag="ob")
                nc.vector.tensor_copy(ob[:], psf[:])
                nc.sync.dma_start(OUT[half * 128 : (half + 1) * 128, :], ob[:])
    nc.finalize()
    return nc


def _pack_weights(w_ih0, w_hh0, b_ih0, b_hh0, w_ih1, w_hh1, b_ih1, b_hh1,
                  fc_w, fc_b):
    CONST = np.zeros((128, 1560), np.float32)
    b0 = (b_ih0 + b_hh0).astype(np.float32)
    b1 = (b_ih1 + b_hh1).astype(np.float32)
    for X, (a, b_) in enumerate(GATE_SLICES):
        CONST[0:64, X * 128 : X * 128 + 64] = w_hh0.T[:, a:b_]
        CONST[0:64, X * 128 + 64 : X * 128 + 128] = w_ih1.T[:, a:b_]
        CONST[64:128, X * 128 + 64 : X * 128 + 128] = w_hh1.T[:, a:b_]
        CONST[0, 512 + X * 128 : 512 + X * 128 + 64] = w_ih0[a:b_, 0]
        CONST[0, 1048 + X * 128 : 1048 + X * 128 + 64] = b0[a:b_]
        CONST[0, 1048 + X * 128 + 64 : 1048 + X * 128 + 128] = b1[a:b_]
    CONST[64:128, 1024:1036] = fc_w.T
    CONST[0, 1036:1048] = fc_b
    return CONST


def kernel(x, w_ih0, w_hh0, b_ih0, b_hh0, w_ih1, w_hh1, b_ih1, b_hh1, fc_w, fc_b):
    x = np.asarray(x, np.float32)
    args = [np.asarray(a, np.float32) for a in (
        w_ih0, w_hh0, b_ih0, b_hh0, w_ih1, w_hh1, b_ih1, b_hh1)]
    fc_w = np.asarray(fc_w, np.float32)
    fc_b = np.asarray(fc_b, np.float32)
    Bx, S, _ = x.shape
    assert Bx == B, f"batch {Bx} != {B}"
    nticks = S + 1

    if nticks not in _BUILD_CACHE:
        _BUILD_CACHE[nticks] = _build(nticks)
    nc = _BUILD_CACHE[nticks]

    CONST = _pack_weights(*args, fc_w, fc_b)
    xT_full = np.zeros((nticks, 2, B), np.float32)
    xT_full[0:S, 0, :] = x[:, :, 0].T
    xT_full[:, 1, :] = 1.0

    in_maps = []
    for c in range(NCORES):
        in_maps.append({
            "xT": np.ascontiguousarray(xT_full[:, :, c * BL : (c + 1) * BL]),
            "CONST": CONST,
        })
    res = bass_utils.run_bass_kernel_spmd(nc, in_maps, core_ids=list(range(NCORES)))
    return np.concatenate([r["out"] for r in res.results], axis=0)
